# revision 4
# baseline (speedup 1.0000x reference)
"""Trainium2 Bass kernel for a 3-layer GCN encoder over two graphs (x, y).

Dense-adjacency formulation, H-stationary orientation:
  GCNConv(h) = D^-1/2 (A+I) D^-1/2 (h @ W) + b
  With Acnt the self-loop-augmented adjacency-count matrix (exact in fp8e4,
  counts <= ~4) and dinv = deg^-1/2:
      H_1     = dinv * x                      (host, shipped bf16)
      P^T_l   = sum_k H_l[k]^T @ A[k, :]      (PE: H k-tile stationary bf16,
                                               A slab moving fp8 -> transposed
                                               partials [f, dst], no transposes)
      zraw_l  = P_l @ W_l + u*b_l  (rank-1)   (PE, psum [dst, f])
      H_l+1   = dinv^2 * relu(zraw_l)         (one scalar-engine activation)
  Layer 3 is transform-first: Y_3 = H_3 @ W_3 per-node before the AllGather
  (halves the layer-3 aggregation width to 128), then
      out = dinv * (P^T_Y3 + b_3*u)^T        (transpose + scaled copy).

Sharding: all 8 cores form one replica group; each core owns a 1280-row
(1250 real) dst shard of BOTH graphs.  A^T is streamed from HBM as fp8
(exact small-integer counts) in chunk-major slabs (20KB DMA descriptors);
H for both graphs is SBUF-resident bf16, replicated with one 8-rank
AllGather per graph per layer boundary (4 total, staggered so each hides
under the other graph's compute).

Node ids are renumbered into a padded space of 10240 = 8*1280 so all tiles
are 128-multiples and the AllGather output is directly the packed SBUF
image of H.
"""

import numpy as np
import ml_dtypes

import concourse.bass as bass
import concourse.tile as tile
from concourse import bacc, mybir
import concourse.bass_utils as bass_utils
from concourse.masks import make_identity

BF16 = ml_dtypes.bfloat16
FP8 = ml_dtypes.float8_e4m3

P = 128          # partitions / tile edge
NC = 8           # cores
N_NODES = 10000
SHARD = 1250     # real nodes per core (per graph)
SHP = 1280       # padded nodes per core
NPAD = NC * SHP  # 10240
KT = NPAD // P   # 80 k-tiles over src nodes
MT = SHP // P    # 10 m-blocks per graph per core
F = 256          # in/hidden feature width
FO = 128         # output feature width
CSZ = 256        # dst chunk size for aggregation slabs
NCH = SHP // CSZ  # 5 chunks per graph

_NC_CACHE = {}


# ----------------------------------------------------------------------------
# Host-side graph preprocessing (index/static work only)
# ----------------------------------------------------------------------------

def _pad_ids(n):
    return (n // SHARD) * SHP + (n % SHARD)


def _prep_graph(x, edge_index, Ws, bs):
    src = edge_index[0].astype(np.int64)
    dst = edge_index[1].astype(np.int64)
    loop = np.arange(N_NODES, dtype=np.int64)
    src = np.concatenate([src, loop])
    dst = np.concatenate([dst, loop])
    sp = _pad_ids(src)
    dp = _pad_ids(dst)

    deg = np.zeros(NPAD, np.float32)
    np.add.at(deg, dp, np.float32(1.0))
    dinv = np.zeros(NPAD, np.float32)
    nz = deg > 0
    dinv[nz] = 1.0 / np.sqrt(deg[nz])
    u = np.sqrt(deg)

    at = np.zeros((NPAD, NPAD), np.float32)   # [src, dst] = A^T counts
    np.add.at(at, (sp, dp), np.float32(1.0))

    h1 = np.zeros((NPAD, F), np.float32)
    h1[_pad_ids(loop)] = x * dinv[_pad_ids(loop)][:, None]
    h1_img = np.ascontiguousarray(
        h1.reshape(KT, P, F).transpose(1, 0, 2).reshape(P, KT * F)
    ).astype(BF16)

    def w_img(W, fo):
        kf = W.shape[0] // P
        return np.ascontiguousarray(
            W.reshape(kf, P, fo).transpose(1, 0, 2).reshape(P, kf * fo)
        ).astype(BF16)

    # per-core A^T slabs: [NCH, P(src in k-tile), KT, CSZ] fp8, 20KB rows
    slabs = []
    for c in range(NC):
        shard = at[:, c * SHP:(c + 1) * SHP]  # [NPAD src, SHP dst]
        slab = np.ascontiguousarray(
            shard.reshape(KT, P, NCH, CSZ).transpose(2, 1, 0, 3)
        ).astype(FP8)
        slabs.append(slab)
    w_imgs = [w_img(Ws[0], F), w_img(Ws[1], F), w_img(Ws[2], FO)]
    b_rows = [bs[0].reshape(1, F).astype(BF16),
              bs[1].reshape(1, F).astype(BF16),
              bs[2].reshape(1, FO).astype(BF16)]
    return slabs, h1_img, w_imgs, b_rows, dinv, u


def prep_in_maps(x, x_edge_index, y, y_edge_index,
                 W1x, b1x, W2x, b2x, W3x, b3x,
                 W1y, b1y, W2y, b2y, W3y, b3y):
    sx, h1x, wx, bx, dx, ux = _prep_graph(
        np.asarray(x, np.float32), np.asarray(x_edge_index),
        (np.asarray(W1x), np.asarray(W2x), np.asarray(W3x)),
        (np.asarray(b1x), np.asarray(b2x), np.asarray(b3x)))
    sy, h1y, wy, by, dy, uy = _prep_graph(
        np.asarray(y, np.float32), np.asarray(y_edge_index),
        (np.asarray(W1y), np.asarray(W2y), np.asarray(W3y)),
        (np.asarray(b1y), np.asarray(b2y), np.asarray(b3y)))
    maps = []
    for c in range(NC):
        dvx = dx[c * SHP:(c + 1) * SHP].reshape(MT, P).T   # [P, MT]
        dvy = dy[c * SHP:(c + 1) * SHP].reshape(MT, P).T
        dv1 = np.concatenate([dvx, dvy], axis=1).astype(np.float32)
        maps.append({
            "at": np.stack([sx[c], sy[c]]),   # [2, NCH, P, KT, CSZ] fp8
            "hx": h1x, "hy": h1y,
            "w0": wx[0], "w1": wx[1], "w2": wx[2],
            "w3": wy[0], "w4": wy[1], "w5": wy[2],
            "b0": bx[0], "b1": bx[1], "b2": bx[2],
            "b3": by[0], "b4": by[1], "b5": by[2],
            "uu": np.concatenate(
                [ux[c * SHP:(c + 1) * SHP], uy[c * SHP:(c + 1) * SHP]]
            ).reshape(1, 2 * SHP).astype(BF16),
            "dv1": np.ascontiguousarray(dv1),                  # [P, 2*MT]
            "dv2": np.ascontiguousarray(dv1 * dv1),            # [P, 2*MT]
        })
    return maps


def _unshard(z_imgs, graph):
    """8 per-core [P, 2*MT*FO] images -> [N_NODES, FO] for graph 0(x)/1(y)."""
    rows = []
    for z in z_imgs:
        zi = z.reshape(P, 2 * MT, FO)[:, graph * MT:(graph + 1) * MT, :]
        r = zi.transpose(1, 0, 2).reshape(SHP, FO)
        rows.append(r[:SHARD])
    return np.concatenate(rows, axis=0)


# ----------------------------------------------------------------------------
# Device kernel
# ----------------------------------------------------------------------------

def _build_nc():
    if "nc" in _NC_CACHE:
        return _NC_CACHE["nc"]
    nc = bacc.Bacc("TRN2", target_bir_lowering=False, debug=False, num_devices=NC)
    dt = mybir.dt

    at = nc.dram_tensor("at", [2, NCH, P, KT, CSZ], dt.float8e4,
                        kind="ExternalInput").ap()
    hx = nc.dram_tensor("hx", [P, KT * F], dt.bfloat16, kind="ExternalInput").ap()
    hy = nc.dram_tensor("hy", [P, KT * F], dt.bfloat16, kind="ExternalInput").ap()
    w_ap = [nc.dram_tensor(f"w{i}", [P, 2 * (FO if i % 3 == 2 else F)], dt.bfloat16,
                           kind="ExternalInput").ap() for i in range(6)]
    b_ap = [nc.dram_tensor(f"b{i}", [1, FO if i % 3 == 2 else F], dt.bfloat16,
                           kind="ExternalInput").ap() for i in range(6)]
    uu = nc.dram_tensor("uu", [1, 2 * SHP], dt.bfloat16, kind="ExternalInput").ap()
    dv1 = nc.dram_tensor("dv1", [P, 2 * MT], dt.float32, kind="ExternalInput").ap()
    dv2 = nc.dram_tensor("dv2", [P, 2 * MT], dt.float32, kind="ExternalInput").ap()
    zout = nc.dram_tensor("z", [P, 2 * MT * FO], dt.float32, kind="ExternalOutput").ap()

    groups = [list(range(NC))]
    Relu = mybir.ActivationFunctionType.Relu
    Copy = mybir.ActivationFunctionType.Copy

    with tile.TileContext(nc) as tc:
        with (
            tc.tile_pool(name="persist", bufs=1) as pers,
            tc.tile_pool(name="aslab", bufs=3) as apool,
            tc.tile_pool(name="work", bufs=3) as wk,
            tc.tile_pool(name="pagg", bufs=2, space="PSUM") as pagg,
            tc.tile_pool(name="pz", bufs=2, space="PSUM") as pzp,
            tc.tile_pool(name="ptr", bufs=2, space="PSUM") as ptr,
            tc.tile_pool(name="dram", bufs=1, space="DRAM") as dp,
        ):
            Hg = [pers.tile([P, KT * F], dt.bfloat16, name="Hx"),
                  pers.tile([P, KT * F], dt.bfloat16, name="Hy")]
            Hown = [pers.tile([P, MT * F], dt.bfloat16, name=f"Hown{g}")
                    for g in range(2)]
            Zsb = pers.tile([P, 2 * MT * FO], dt.float32)
            Wt = [pers.tile([P, 2 * (FO if i % 3 == 2 else F)], dt.bfloat16,
                            name=f"wt{i}") for i in range(6)]
            Bt = [pers.tile([1, FO if i % 3 == 2 else F], dt.bfloat16, name=f"bt{i}")
                  for i in range(6)]
            Ut = pers.tile([1, 2 * SHP], dt.bfloat16)
            Dv1 = pers.tile([P, 2 * MT], dt.float32)
            Dv2 = pers.tile([P, 2 * MT], dt.float32)
            ident = pers.tile([P, P], dt.bfloat16)

            make_identity(nc, ident[:])
            # chunked initial H loads so layer-1 matmuls can start early
            CH = KT * F // 4
            for r in range(4):
                nc.sync.dma_start(Hg[0][:, r * CH:(r + 1) * CH],
                                  hx[:, r * CH:(r + 1) * CH])
            for r in range(4):
                nc.sync.dma_start(Hg[1][:, r * CH:(r + 1) * CH],
                                  hy[:, r * CH:(r + 1) * CH])
            for i in range(6):
                nc.sync.dma_start(Wt[i][:], w_ap[i])
                nc.sync.dma_start(Bt[i][:], b_ap[i])
            nc.sync.dma_start(Ut[:], uu)
            nc.sync.dma_start(Dv1[:], dv1)
            nc.sync.dma_start(Dv2[:], dv2)

            for layer in range(3):
                fw = FO if layer == 2 else F     # aggregation feature width
                nf = fw // P                     # f-halves (2 or 1)
                for g in range(2):
                    Wl = Wt[3 * g + layer]
                    Bl = Bt[3 * g + layer]
                    W3l = Wt[3 * g + 2]
                    for ch in range(NCH):
                        slab = apool.tile([P, KT * CSZ], dt.float8e4, tag="aslab")
                        nc.scalar.dma_start(slab[:], at[g, ch])
                        pps = []
                        for f in range(nf):
                            pp = pagg.tile([P, CSZ], dt.float32, tag=f"agg{f}")
                            for k in range(KT):
                                nc.tensor.matmul(
                                    pp[:],
                                    lhsT=Hg[g][:, k * fw + f * P:
                                               k * fw + (f + 1) * P],
                                    rhs=slab[:, k * CSZ:(k + 1) * CSZ],
                                    start=(k == 0),
                                    stop=(layer < 2 and k == KT - 1),
                                )
                            if layer == 2:
                                # bias folded pre-transpose: pp += b3 (x) u
                                nc.tensor.matmul(
                                    pp[:],
                                    lhsT=Bl[:1, :FO],
                                    rhs=Ut[:1, g * SHP + ch * CSZ:
                                           g * SHP + (ch + 1) * CSZ],
                                    start=False,
                                    stop=True,
                                )
                            pps.append(pp)
                        PTs = wk.tile([P, nf * CSZ], dt.bfloat16, tag="pts")
                        for f in range(nf):
                            nc.vector.tensor_copy(
                                PTs[:, f * CSZ:(f + 1) * CSZ], pps[f][:])
                        for s in range(CSZ // P):
                            blk = ch * (CSZ // P) + s
                            gm = g * MT + blk
                            if layer < 2:
                                z = pzp.tile([P, F], dt.float32, tag="z")
                                for f in range(2):
                                    nc.tensor.matmul(
                                        z[:],
                                        lhsT=PTs[:, f * CSZ + s * P:
                                                 f * CSZ + (s + 1) * P],
                                        rhs=Wl[:, f * F:(f + 1) * F],
                                        start=(f == 0),
                                        stop=False,
                                    )
                                nc.tensor.matmul(
                                    z[:],
                                    lhsT=Ut[:1, g * SHP + blk * P:
                                            g * SHP + (blk + 1) * P],
                                    rhs=Bl[:1, :F],
                                    start=False,
                                    stop=True,
                                )
                                if layer == 0:
                                    nc.scalar.activation(
                                        Hown[g][:, blk * F:(blk + 1) * F],
                                        z[:], Relu, scale=Dv2[:, gm:gm + 1])
                                else:
                                    # H3 block, then Y3 = H3 @ W3 (transform-
                                    # first: halves layer-3 aggregation width)
                                    Xb = wk.tile([P, F], dt.bfloat16, tag="xb")
                                    nc.scalar.activation(
                                        Xb[:], z[:], Relu,
                                        scale=Dv2[:, gm:gm + 1])
                                    X3T = wk.tile([P, F], dt.bfloat16, tag="x3t")
                                    for fc in range(2):
                                        tp_ = ptr.tile([P, P], dt.bfloat16,
                                                       tag="tr")
                                        nc.tensor.transpose(
                                            tp_[:], Xb[:, fc * P:(fc + 1) * P],
                                            ident[:])
                                        nc.vector.tensor_copy(
                                            X3T[:, fc * P:(fc + 1) * P], tp_[:])
                                    zy = pzp.tile([P, F], dt.float32, tag="z")
                                    for fc in range(2):
                                        nc.tensor.matmul(
                                            zy[:, :FO],
                                            lhsT=X3T[:, fc * P:(fc + 1) * P],
                                            rhs=W3l[:, fc * FO:(fc + 1) * FO],
                                            start=(fc == 0),
                                            stop=(fc == 1),
                                        )
                                    nc.vector.tensor_copy(
                                        Hown[g][:, blk * FO:(blk + 1) * FO],
                                        zy[:, :FO])
                            else:
                                tp_ = ptr.tile([P, P], dt.bfloat16, tag="tr")
                                nc.tensor.transpose(
                                    tp_[:], PTs[:, s * P:(s + 1) * P], ident[:])
                                nc.scalar.activation(
                                    Zsb[:, gm * FO:(gm + 1) * FO],
                                    tp_[:], Copy, scale=Dv1[:, gm:gm + 1])
                    if layer < 2:
                        # AllGather this graph's new own-shard into its H image
                        wown = MT * (F if layer == 0 else FO)
                        agin = dp.tile([P, wown], dt.bfloat16,
                                       tag=f"agin{layer}{g}")
                        agout = dp.tile([NC * P, wown], dt.bfloat16,
                                        tag=f"agout{layer}{g}")
                        nc.sync.dma_start(agin[:], Hown[g][:, :wown])
                        nc.gpsimd.collective_compute(
                            "AllGather",
                            mybir.AluOpType.bypass,
                            replica_groups=groups,
                            ins=[agin[:].opt()],
                            outs=[agout[:].opt()],
                        )
                        for r in range(NC):
                            nc.gpsimd.dma_start(
                                Hg[g][:, r * wown:(r + 1) * wown],
                                agout[r * P:(r + 1) * P, :],
                            )
            nc.sync.dma_start(zout, Zsb[:])
    nc.compile()
    _NC_CACHE["nc"] = nc
    return nc


# ----------------------------------------------------------------------------
# Entry point
# ----------------------------------------------------------------------------

def kernel(x, x_edge_index, y, y_edge_index,
           W1x, b1x, W2x, b2x, W3x, b3x,
           W1y, b1y, W2y, b2y, W3y, b3y,
           _trace=False, _trace_cores=None):
    in_maps = prep_in_maps(x, x_edge_index, y, y_edge_index,
                           W1x, b1x, W2x, b2x, W3x, b3x,
                           W1y, b1y, W2y, b2y, W3y, b3y)
    nc = _build_nc()
    kw = {}
    if _trace:
        kw = dict(trace=True, trace_cores=_trace_cores or [0])
    res = bass_utils.run_bass_kernel_spmd(
        nc, in_maps, core_ids=list(range(NC)), **kw
    )
    z = [res.results[c]["z"] for c in range(NC)]
    out_x = _unshard(z, 0)
    out_y = _unshard(z, 1)
    if _trace:
        kernel._last_result = res
    return out_x, out_y
